# revision 1
# baseline (speedup 1.0000x reference)
"""Trainium2 Bass kernel: causal multi-head attention with an extra time-mixing
matrix D (attn = D @ softmax(mask(Q K^T / sqrt(e))) @ V, concat heads, out proj).

Shapes (hardcoded): B=4, T=2048, d=1024, H=16, e=64, fp32.
Sharding over 8 NeuronCores: data-parallel over batch (4) x tensor-parallel over
heads (2 groups of 8). Each core computes its batch/head-group partial
y_part = concat(attn_heads) @ Wo_part^T; host sums the 2 partials per batch and
adds bo.

All heavy matmuls run as float32r (hw splits fp32 into bf16 hi/lo products:
~1e-4 rel err, full bf16-rate throughput at moving dim >= 256).
"""

import sys

for _p in ("/opt/trn_rl_repo", "/root/.axon_site/_ro/trn_rl_repo"):
    if _p not in sys.path:
        sys.path.append(_p)

from contextlib import ExitStack

import numpy as np

import concourse.bass as bass  # noqa: F401  (AP helpers)
import concourse.tile as tile
from concourse import bacc, mybir
from concourse.bass_utils import run_bass_kernel_spmd

dt = mybir.dt

B, T, D, H, E = 4, 2048, 1024, 16, 64
HG = 8          # heads per core (tensor-parallel group)
COEF = 1.0 / E ** 0.5
P = 128         # partitions
TQB = 512       # query-block width
NTQ = T // TQB  # 4 query blocks
NTC = T // P    # 16 time chunks
ND = D // P     # 8 contraction chunks (d)

_CACHED_NC = None


def _build_nc():
    """Build + compile the single-core program (same NEFF on all 8 cores)."""
    import os
    _phase = os.environ.get("KPHASE", "all")
    nc = bacc.Bacc("TRN2", target_bir_lowering=False, debug=False)
    f32, f32r = dt.float32, dt.float32r
    Exp = mybir.ActivationFunctionType.Exp
    mult = mybir.AluOpType.mult

    xqT = nc.dram_tensor("xqT", [D + 1, T], f32r, kind="ExternalInput").ap()
    xkT = nc.dram_tensor("xkT", [D + 1, T], f32r, kind="ExternalInput").ap()
    xvT = nc.dram_tensor("xvT", [D + 1, T], f32r, kind="ExternalInput").ap()
    wqT = nc.dram_tensor("wqT", [D + 1, 512], f32r, kind="ExternalInput").ap()
    wkT = nc.dram_tensor("wkT", [D + 1, 512], f32r, kind="ExternalInput").ap()
    wvT = nc.dram_tensor("wvT", [D + 1, 512], f32r, kind="ExternalInput").ap()
    woT = nc.dram_tensor("woT", [512, D], f32r, kind="ExternalInput").ap()
    dT = nc.dram_tensor("dT", [T, T], f32r, kind="ExternalInput").ap()
    msk = nc.dram_tensor("msk", [512, 1024], f32, kind="ExternalInput").ap()
    idn = nc.dram_tensor("idn", [P, P], f32, kind="ExternalInput").ap()
    y = nc.dram_tensor("y", [T, D], f32, kind="ExternalOutput").ap()

    with tile.TileContext(nc) as tc, ExitStack() as ctx:
        # ---- constants + persistent results --------------------------------
        consts = ctx.enter_context(tc.tile_pool(name="consts", bufs=1))
        idt = consts.tile([P, P], f32, tag="idt")
        nc.sync.dma_start(idt[:], idn[:])
        mskt = consts.tile([P, 4 * 1024], f32, tag="mskt")
        for j in range(4):
            nc.sync.dma_start(mskt[:, 1024 * j:1024 * (j + 1)],
                              msk[P * j:P * (j + 1), :])
        ones32 = consts.tile([P, HG], f32, tag="ones32")
        nc.vector.memset(ones32[:], 1.0)
        onesr = consts.tile([P, HG], f32r, tag="onesr")
        nc.vector.tensor_copy(onesr[:], ones32[:])

        proj = ctx.enter_context(tc.tile_pool(name="proj", bufs=1))
        qt = [proj.tile([P, T], f32r, tag=f"qt{p}", name=f"qt{p}") for p in range(4)]
        kt = [proj.tile([P, T], f32r, tag=f"kt{p}", name=f"kt{p}") for p in range(4)]
        vt = [proj.tile([P, HG * (E + 1)], f32r, tag=f"vt{t}", name=f"vt{t}") for t in range(NTC)]
        for t in range(NTC):
            ones_dst = vt[t][:].rearrange("p (h c) -> p h c", c=E + 1)[:, :, E]
            nc.vector.tensor_copy(ones_dst, onesr[:])

        # ---- projections ---------------------------------------------------
        def load_w(pool, wdram, name):
            tiles = []
            for d in range(ND + 1):
                rows = P if d < ND else 1
                w = pool.tile([P, 512], f32r, tag=f"{name}{d}", name=f"{name}{d}")
                nc.sync.dma_start(w[0:rows, :], wdram[P * d:P * d + rows, :])
                tiles.append(w)
            return tiles

        def load_x_block(pool, xdram, tb, tag):
            xb = pool.tile([P, (ND + 1) * 512], f32r, tag=tag)
            for d in range(ND + 1):
                rows = P if d < ND else 1
                nc.sync.dma_start(xb[0:rows, 512 * d:512 * d + 512],
                                  xdram[P * d:P * d + rows, TQB * tb:TQB * (tb + 1)])
            return xb

        # V projection: psum [t 128, 8 heads x 64] per t-chunk
        xs_stack = ExitStack()
        xs_pool = xs_stack.enter_context(tc.tile_pool(name="xs", bufs=3))
        with tc.tile_pool(name="wv", bufs=1) as wpool, \
             tc.tile_pool(name="psv", bufs=6, space="PSUM") as pspool:
            xpool = xs_pool
            wv = load_w(wpool, wvT, "wv")
            for tb in range(NTQ):
                xb = load_x_block(xpool, xvT, tb, "xs")
                for tc_ in range(4):
                    ps = pspool.tile([P, 512], dt.float32, tag="psv")
                    for d in range(ND + 1):
                        rows = P if d < ND else 1
                        nc.tensor.matmul(
                            ps[:],
                            xb[0:rows, 512 * d + P * tc_:512 * d + P * (tc_ + 1)],
                            wv[d][0:rows, :],
                            start=(d == 0), stop=(d == ND))
                    t = 4 * tb + tc_
                    dst = vt[t][:].rearrange("p (h c) -> p h c", c=E + 1)[:, :, 0:E]
                    src = ps[:].rearrange("p (h c) -> p h c", c=E)
                    nc.vector.tensor_copy(dst, src)

        # Q/K projections: psum [e2 128 (head pair), t 512] per (pair, t-block)
        def qk_proj(xdram, wdram, dest, name):
            with tc.tile_pool(name=f"w{name}", bufs=1) as wpool, \
                 tc.tile_pool(name=f"ps{name}", bufs=6, space="PSUM") as pspool:
                xpool = xs_pool
                wt = load_w(wpool, wdram, name)
                for tb in range(NTQ):
                    xb = load_x_block(xpool, xdram, tb, "xs")
                    for p in range(4):
                        ps = pspool.tile([P, 512], dt.float32, tag=f"ps{name}")
                        for d in range(ND + 1):
                            rows = P if d < ND else 1
                            nc.tensor.matmul(
                                ps[:],
                                wt[d][0:rows, P * p:P * (p + 1)],
                                xb[0:rows, 512 * d:512 * d + 512],
                                start=(d == 0), stop=(d == ND))
                        nc.vector.tensor_copy(dest[p][:, TQB * tb:TQB * (tb + 1)], ps[:])

        if _phase != "v":
            qk_proj(xqT, wqT, qt, "q")
            qk_proj(xkT, wkT, kt, "k")

        xs_stack.close()

        # ---- attention: scores^T -> exp -> (mask) -> PV^T + rowsums --------
        with tc.tile_pool(name="pvg", bufs=1) as pvgp:
          if _phase in ("v", "proj"):
            pass
          else:
           pvg = [pvgp.tile([P, 512], f32r, tag=f"pvg{t}", name=f"pvg{t}") for t in range(NTC)]
           with tc.tile_pool(name="sps", bufs=2, space="PSUM") as sps, \
              tc.tile_pool(name="pvps", bufs=3, space="PSUM") as pvps, \
              tc.tile_pool(name="tps", bufs=1, space="PSUM") as tps, \
              tc.tile_pool(name="upool", bufs=10) as upool, \
              tc.tile_pool(name="npool", bufs=3) as npool:
             for p in range(4):
                 for i in range(NTQ):
                     nch = 4 * (i + 1)
                     pvA = pvps.tile([P, 512], dt.float32, tag="pv")
                     pvB = pvps.tile([P, 512], dt.float32, tag="pv")
                     for c in range(nch):
                         sp = sps.tile([P, 1024], dt.float32, tag="sp")
                         for h in range(2):  # row-packed pair: K=64 each
                             nc.tensor.matmul(
                                 sp[:, 512 * h:512 * (h + 1)],
                                 kt[p][64 * h:64 * (h + 1), P * c:P * (c + 1)],
                                 qt[p][64 * h:64 * (h + 1), TQB * i:TQB * (i + 1)],
                                 start=True, stop=True)
                         u = upool.tile([P, 1024], f32r, tag="u")
                         nc.scalar.activation(u[:], sp[:], Exp, scale=COEF)
                         j = c - (nch - 4)
                         if j >= 0:
                             nc.vector.tensor_tensor(
                                 u[:], u[:], mskt[:, 1024 * j:1024 * (j + 1)], op=mult)
                         for h, pv in ((0, pvA), (1, pvB)):
                             nc.tensor.matmul(
                                 pv[0:E + 1, :],
                                 vt[c][:, (E + 1) * (2 * p + h):(E + 1) * (2 * p + h + 1)],
                                 u[:, 512 * h:512 * (h + 1)],
                                 start=(c == 0), stop=(c == nch - 1))
                     # normalize + transpose into pvg (natural [t, head cols])
                     for h, pv in ((0, pvA), (1, pvB)):
                         tmp = npool.tile([P, 512], dt.float32, tag="ntmp")
                         nc.vector.tensor_copy(tmp[0:E, :], pv[0:E, :])
                         nc.vector.reciprocal(tmp[E:E + 1, :], pv[E:E + 1, :])
                         col = E * (2 * p + h)
                         for qs in range(4):
                             tp = tps.tile([P, P], dt.float32, tag="tp")
                             nc.tensor.transpose(
                                 tp[0:P, 0:E + 1],
                                 tmp[0:E + 1, P * qs:P * (qs + 1)],
                                 idt[0:E + 1, 0:E + 1])
                             nc.vector.tensor_scalar(
                                 pvg[4 * i + qs][:, col:col + E],
                                 tp[:, 0:E], tp[:, E:E + 1], None, op0=mult)

          # ---- attn = D @ PV (transposed), then out projection ------------
          if _phase not in ("attn", "v", "proj"):
            with tc.tile_pool(name="dtp", bufs=10) as dtp, \
                 tc.tile_pool(name="wo", bufs=1) as wop, \
                 tc.tile_pool(name="a2s", bufs=10) as a2sp, \
                 tc.tile_pool(name="obuf", bufs=3) as obp, \
                 tc.tile_pool(name="aps", bufs=6, space="PSUM") as aps, \
                 tc.tile_pool(name="ops", bufs=2, space="PSUM") as ops:
                wo = []
                for cc in range(4):
                    w = wop.tile([P, D], f32r, tag=f"wo{cc}")
                    nc.sync.dma_start(w[:], woT[P * cc:P * (cc + 1), :])
                    wo.append(w)
                for qb in range(NTQ):
                    a2 = [aps.tile([P, 512], dt.float32, tag="a2", name="a2") for _ in range(4)]
                    for t in range(NTC):
                        dtt = dtp.tile([P, 512], f32r, tag="dt")
                        nc.sync.dma_start(
                            dtt[:], dT[P * t:P * (t + 1), TQB * qb:TQB * (qb + 1)])
                        for p in range(4):
                            nc.tensor.matmul(
                                a2[p][:], pvg[t][:, P * p:P * (p + 1)], dtt[:],
                                start=(t == 0), stop=(t == NTC - 1))
                    a2s = []
                    for p in range(4):
                        s = a2sp.tile([P, 512], f32r, tag="a2s", name="a2s")
                        nc.vector.tensor_copy(s[:], a2[p][:])
                        a2s.append(s)
                    for qs in range(4):
                        for nh in range(2):
                            op_ = ops.tile([P, 512], dt.float32, tag="op")
                            for cc in range(4):
                                nc.tensor.matmul(
                                    op_[:],
                                    a2s[cc][:, P * qs:P * (qs + 1)],
                                    wo[cc][:, 512 * nh:512 * (nh + 1)],
                                    start=(cc == 0), stop=(cc == 3))
                            ob = obp.tile([P, 512], dt.float32, tag="ob")
                            nc.vector.tensor_copy(ob[:], op_[:])
                            nc.sync.dma_start(
                                y[TQB * qb + P * qs:TQB * qb + P * (qs + 1),
                                  512 * nh:512 * (nh + 1)],
                                ob[:])

    nc.compile()
    return nc


def _prep_inputs(query_1, key_1, value_1, Wq, bq, Wk, bk, Wv, bv, Wo, bo, Dmat):
    """Host-side sharding: per-core input dicts."""
    f = np.float32
    ones_row = np.ones((1, T), f)

    def xT(x, b):
        return np.ascontiguousarray(np.vstack([np.asarray(x[b], f).T, ones_row]))

    # per head-group weights
    wqTs, wkTs, wvTs, woTs = [], [], [], []
    for g in range(2):
        h0 = HG * g
        wq = np.zeros((D + 1, 512), f)
        wk = np.zeros((D + 1, 512), f)
        for p in range(4):
            for h in range(2):
                hh = h0 + 2 * p + h
                c0 = 128 * p + 64 * h
                wq[:D, c0:c0 + 64] = np.asarray(Wq[hh], f).T
                wq[D, c0:c0 + 64] = np.asarray(bq[hh], f)
                wk[:D, c0:c0 + 64] = np.asarray(Wk[hh], f).T
                wk[D, c0:c0 + 64] = np.asarray(bk[hh], f)
        wv = np.zeros((D + 1, 512), f)
        for j in range(HG):
            wv[:D, 64 * j:64 * (j + 1)] = np.asarray(Wv[h0 + j], f).T
            wv[D, 64 * j:64 * (j + 1)] = np.asarray(bv[h0 + j], f)
        wo = np.ascontiguousarray(np.asarray(Wo, f)[:, 64 * h0:64 * (h0 + HG)].T)
        wqTs.append(wq); wkTs.append(wk); wvTs.append(wv); woTs.append(wo)

    dT = np.ascontiguousarray(np.asarray(Dmat, f).T)
    r = np.arange(P)[None, :, None]
    jj = (128 * np.arange(4))[:, None, None]
    s = np.arange(512)[None, None, :]
    m = ((jj + r) <= s).astype(f)                       # [4, 128, 512]
    msk = np.ascontiguousarray(np.tile(m, (1, 1, 2)).reshape(512, 1024))
    idn = np.eye(P, dtype=f)

    xqTs = [xT(query_1, b) for b in range(B)]
    xkTs = [xT(key_1, b) for b in range(B)]
    xvTs = [xT(value_1, b) for b in range(B)]

    in_maps = []
    for c in range(8):
        b, g = c // 2, c % 2
        in_maps.append({
            "xqT": xqTs[b], "xkT": xkTs[b], "xvT": xvTs[b],
            "wqT": wqTs[g], "wkT": wkTs[g], "wvT": wvTs[g], "woT": woTs[g],
            "dT": dT, "msk": msk, "idn": idn,
        })
    return in_maps


def kernel(query_1, key_1, value_1, Wq, bq, Wk, bk, Wv, bv, Wo, bo, D):
    import os
    os.environ["BASS_NEVER_TRACE"] = "1"  # NTFF capture hangs over the axon relay
    global _CACHED_NC
    if _CACHED_NC is None:
        _CACHED_NC = _build_nc()
    nc = _CACHED_NC
    in_maps = _prep_inputs(query_1, key_1, value_1, Wq, bq, Wk, bk, Wv, bv, Wo, bo, D)
    res = run_bass_kernel_spmd(nc, in_maps, core_ids=list(range(8)))
    bo_f = np.asarray(bo, np.float32)
    out = np.empty((B, T, 1024), np.float32)
    for b in range(B):
        out[b] = res.results[2 * b]["y"] + res.results[2 * b + 1]["y"] + bo_f
    return out

